# revision 11
# baseline (speedup 1.0000x reference)
"""Boolean reservoir kernel for Trainium2 (8 NeuronCores, data-parallel over samples).

Reference computation (per sample m):
    res[input_nodes] = x[t]                      (scatter input bits)
    idx[n] = sum_k res[k] * primes[k] * W[n,k]   (masked prime-weighted sum)
    res    = lut[n, idx[n]]                      (per-node LUT bit lookup)
    ... 512 sequential steps, then readout = res @ readout_w.T + b

Kernel formulation (v2 — dma_gather):
  - Nodes are permuted (input nodes last); node n' = 2p+h lives on SBUF
    partition p, column half h.  PW rows for input nodes are zeroed and their
    contribution precomputed on host per (t, m, n) as "xc" (exact fp32).
  - idx < 2^17 for this problem (max_n sum_k primes[k] W[n,k] = 111381, checked
    at prep).  The LUT is repacked host-side into a windowed table of
    32768 rows x 256B: row n'*128 + w holds bits [1024w, 1024w+2048) of node
    n''s LUT.  Row index = n'*128 + (idx>>10) fits int16; the target bit is
    always within the row's first 32 words (word (idx>>5)&31, bit idx&31).
  - Per step each cohort (16 of the core's 32 samples) issues ONE
    InstDMAGatherAnt for all 4096 (node, sample) pairs — vs 64 SWDGE
    indirect DMAs in v1 (994ns fixed overhead each).  The int16 row indices
    are shuffled into the gather's wrapped layout ([i%16, i//16] for gather
    i = g*128+p) via a DRAM bounce (fancy AP on the DRAM side only) plus an
    on-chip restride.  The selected word is extracted with exact int ops
    (shift / is_equal on {0,1} / masked sum) on DVE.
  - Two cohorts pipeline: cohort B's matmul/select overlaps cohort A's gather.
"""

import sys

for _p in ("/opt/trn_rl_repo", "/root/.axon_site/_ro/trn_rl_repo"):
    if _p not in sys.path:
        sys.path.insert(0, _p)

import numpy as np

import concourse.bacc as bacc
import concourse.bass as bass
import concourse.mybir as mybir
import concourse.tile as tile
from concourse import library_config
from concourse.alu_op_type import AluOpType
from concourse.bass_utils import run_bass_kernel_spmd

# Problem dims (hardcoded per spec)
R = 256
M = 256
S = 512
NB = 32            # d*b input bits
OUT = 32
NCORES = 8
MLOC = M // NCORES          # 32 samples per core
C = 4                       # cohorts per core (pipelined)
JC = MLOC // C              # 16 samples per cohort
FBC = 2 * JC                # 32 cols per cohort tile (h*JC + j)
NIDX = 128 * FBC            # 4096 gathers per cohort-step
WSEL = 32                   # word-select width
WSHIFT = 10                 # idx window shift (idx < 2^17)
NWIN = 128                  # windows per node
NROWS = R * NWIN            # 32768 table rows
EW = 64                     # int32 words per row (256B)
XC_CHUNK = 8                # steps of xc prefetched per DMA
P_FULL = 128

F32 = mybir.dt.float32
I32 = mybir.dt.int32
I16 = mybir.dt.int16


def build_bass(steps: int = S):
    """Build + compile the (input-independent) bass program."""
    nc = bacc.Bacc(
        "TRN2",
        target_bir_lowering=False,
        debug=False,
        enable_asserts=False,
        num_devices=NCORES,
        num_swdge_queues=4,
    )

    tab_d = nc.dram_tensor("tab", [NROWS, EW], I32, kind="ExternalInput")
    pw_d = nc.dram_tensor("pw", [2, 2, P_FULL, P_FULL], F32, kind="ExternalInput")
    rwt_d = nc.dram_tensor("rwt", [2, P_FULL, OUT], F32, kind="ExternalInput")
    bias_d = nc.dram_tensor("bias", [JC, OUT], F32, kind="ExternalInput")
    baseg_d = nc.dram_tensor("baseg", [P_FULL, FBC], I32, kind="ExternalInput")
    iota_d = nc.dram_tensor("iota", [P_FULL, FBC * WSEL], I32, kind="ExternalInput")
    res0_d = nc.dram_tensor("res0", [P_FULL, C * FBC], F32, kind="ExternalInput")
    nxc = (steps + XC_CHUNK - 1) // XC_CHUNK
    xc_d = nc.dram_tensor("xc", [nxc, P_FULL, XC_CHUNK * C * FBC], F32,
                          kind="ExternalInput")
    scr_d = nc.dram_tensor("scr", [C, 16, 8 * FBC], I16, kind="Internal")
    out_d = nc.dram_tensor("out", [MLOC, OUT], F32, kind="ExternalOutput")

    tab_ap = tab_d.ap()

    with tile.TileContext(nc) as tc:
        with (
            tc.tile_pool(name="const", bufs=1) as cpool,
            tc.tile_pool(name="state", bufs=1) as spool,
            tc.tile_pool(name="work", bufs=3) as wpool,
            tc.tile_pool(name="gath", bufs=2) as gpool,
            tc.tile_pool(name="xc", bufs=2) as xcpool,
            tc.tile_pool(name="psum", bufs=2, space="PSUM") as ppool,
        ):
            nc.gpsimd.load_library(library_config.mlp)

            # --- constants to SBUF ---
            pw_t = [[cpool.tile([P_FULL, P_FULL], F32, tag=f"pw{ho}{hk}",
                                name=f"pw{ho}{hk}")
                     for hk in range(2)] for ho in range(2)]
            for ho in range(2):
                for hk in range(2):
                    nc.sync.dma_start(out=pw_t[ho][hk][:], in_=pw_d.ap()[ho, hk])
            rwt_t = [cpool.tile([P_FULL, OUT], F32, tag=f"rwt{h}", name=f"rwt{h}")
                     for h in range(2)]
            for h in range(2):
                nc.sync.dma_start(out=rwt_t[h][:], in_=rwt_d.ap()[h])
            bias_t = cpool.tile([JC, OUT], F32, tag="bias", name="biast")
            nc.sync.dma_start(out=bias_t[:], in_=bias_d.ap())
            baseg_t = cpool.tile([P_FULL, FBC], I32, tag="baseg", name="basegt")
            nc.sync.dma_start(out=baseg_t[:], in_=baseg_d.ap())
            iota_t = cpool.tile([P_FULL, FBC, WSEL], I32, tag="iota", name="iotat")
            nc.sync.dma_start(
                out=iota_t[:],
                in_=iota_d.ap().rearrange("p (g w) -> p g w", w=WSEL),
            )
            wsh_t = cpool.tile([P_FULL, 1], I32, tag="wsh", name="wsht")
            nc.vector.memset(wsh_t[:], WSHIFT)
            one_t = cpool.tile([P_FULL, 1], I32, tag="one", name="onet")
            nc.vector.memset(one_t[:], 1)

            # --- persistent state per cohort ---
            res_t = [spool.tile([P_FULL, FBC], F32, tag=f"res{c}", name=f"res{c}")
                     for c in range(C)]
            idxs_t = [spool.tile([P_FULL, 8 * FBC], I16, tag=f"idxs{c}",
                                 name=f"idxs{c}") for c in range(C)]
            for c in range(C):
                nc.sync.dma_start(
                    out=res_t[c][:],
                    in_=res0_d.ap()[:, c * FBC:(c + 1) * FBC],
                )
                nc.vector.memset(idxs_t[c][:], 0)

            xct = None
            for t in range(steps):
                ct, ti = divmod(t, XC_CHUNK)
                if ti == 0:
                    xct = xcpool.tile([P_FULL, XC_CHUNK * C * FBC], F32,
                                      tag="xc", name="xct")
                    nc.sync.dma_start(out=xct[:], in_=xc_d.ap()[ct])

                for c in range(C):
                    res = res_t[c]
                    idxs16 = idxs_t[c]

                    # state matmul: psum[p, ho*JC+j] = idx contribution
                    psum = ppool.tile([P_FULL, FBC], F32, space="PSUM",
                                      tag="mm", name=f"psum{c}")
                    for ho in range(2):
                        for hk in range(2):
                            nc.tensor.matmul(
                                psum[:, ho * JC:(ho + 1) * JC],
                                pw_t[ho][hk][:],
                                res[:, hk * JC:(hk + 1) * JC],
                                start=(hk == 0),
                                stop=(hk == 1),
                            )

                    xc_sl = xct[:, (ti * C + c) * FBC:(ti * C + c + 1) * FBC]

                    # idx = int32(psum + xc)   (exact: values < 2^17)
                    idx = wpool.tile([P_FULL, FBC], I32, tag=f"idx{c}",
                                     name=f"idx{c}")
                    nc.vector.scalar_tensor_tensor(
                        out=idx[:], in0=psum[:], scalar=1.0, in1=xc_sl,
                        op0=AluOpType.mult, op1=AluOpType.add,
                    )
                    # row index = (idx >> 10) | n'*128 (OR == add: idx>>10 < 128)
                    blk32 = wpool.tile([P_FULL, FBC], I32, tag=f"bk32{c}",
                                       name=f"bk32{c}")
                    nc.vector.scalar_tensor_tensor(
                        out=blk32[:], in0=idx[:], scalar=wsh_t[:, :1],
                        in1=baseg_t[:],
                        op0=AluOpType.logical_shift_right,
                        op1=AluOpType.bitwise_or,
                    )
                    blk16 = wpool.tile([P_FULL, FBC], I16, tag=f"blk{c}",
                                       name=f"blk{c}")
                    nc.vector.tensor_copy(out=blk16[:], in_=blk32[:])
                    # shuffle into wrapped idx layout via DRAM bounce:
                    # scr[c][r, a*FBC+g] = blk16[16a+r, g]
                    nc.sync.dma_start(
                        out=scr_d.ap()[c].rearrange("r (a g) -> a r g", a=8),
                        in_=blk16[:, :],
                    )
                    tmp16 = wpool.tile([P_FULL, 8 * FBC], I16, tag=f"tmp{c}",
                                       name=f"tmp{c}")
                    nc.sync.dma_start(
                        out=tmp16[:, :],
                        in_=scr_d.ap()[c].unsqueeze(0)
                            .broadcast_to((8, 16, 8 * FBC)),
                    )
                    # restride "(a g)" -> "(g a)" on all bands (idxs
                    # replicated per 16-partition band for the Q7 cores)
                    nc.vector.tensor_copy(
                        out=idxs16[:, :].rearrange("m (g a) -> m g a", a=8),
                        in_=tmp16[:, :].rearrange("m (a g) -> m a g", a=8)
                            .transpose([0, 2, 1]),
                    )

                    # two 1024-idx gathers (SWDGE ring limit), rotating queues
                    words = gpool.tile([P_FULL, FBC, EW], I32, tag=f"w{c}",
                                       name=f"w{c}")
                    NH = NIDX // 2
                    for hf in range(2):
                        nc.gpsimd.dma_gather(
                            words[:, (FBC // 2) * hf:(FBC // 2) * (hf + 1), :],
                            tab_ap,
                            idxs16[:, (NH // 16) * hf:(NH // 16) * (hf + 1)],
                            NH, NH, EW, queue_num=(2 * c + hf) % 4,
                        )

                    # word select (s = (idx>>5)&31) + bit select (idx&31),
                    # all ops exact (shifts int-domain; mult/add on {0,1})
                    s_t = wpool.tile([P_FULL, FBC], I32, tag=f"s{c}",
                                     name=f"s{c}")
                    nc.vector.tensor_scalar(
                        out=s_t[:], in0=idx[:], scalar1=5, scalar2=31,
                        op0=AluOpType.logical_shift_right,
                        op1=AluOpType.bitwise_and,
                    )
                    b5 = wpool.tile([P_FULL, FBC], I32, tag=f"b5{c}",
                                    name=f"b5{c}")
                    nc.vector.tensor_scalar(
                        out=b5[:], in0=idx[:], scalar1=31, scalar2=None,
                        op0=AluOpType.bitwise_and,
                    )
                    bits = wpool.tile([P_FULL, FBC, WSEL], I32, tag=f"bits{c}",
                                      name=f"bits{c}")
                    nc.vector.tensor_tensor(
                        out=bits[:], in0=words[:, :, 0:WSEL],
                        in1=b5[:, :].unsqueeze(2).broadcast_to(
                            (P_FULL, FBC, WSEL)),
                        op=AluOpType.logical_shift_right,
                    )
                    m01 = wpool.tile([P_FULL, FBC, WSEL], I32, tag=f"m01{c}",
                                     name=f"m01{c}")
                    nc.vector.tensor_tensor(
                        out=m01[:], in0=iota_t[:],
                        in1=s_t[:, :].unsqueeze(2).broadcast_to(
                            (P_FULL, FBC, WSEL)),
                        op=AluOpType.is_equal,
                    )
                    prod = wpool.tile([P_FULL, FBC, WSEL], I32, tag=f"prod{c}",
                                      name=f"prod{c}")
                    nc.vector.scalar_tensor_tensor(
                        out=prod[:], in0=bits[:], scalar=one_t[:, :1],
                        in1=m01[:],
                        op0=AluOpType.bitwise_and, op1=AluOpType.bitwise_and,
                    )
                    bi = wpool.tile([P_FULL, FBC], I32, tag=f"bi{c}",
                                    name=f"bi{c}")
                    with nc.allow_low_precision(reason="0/1 sums are exact"):
                        nc.vector.tensor_reduce(
                            out=bi[:], in_=prod[:],
                            axis=mybir.AxisListType.X, op=AluOpType.add,
                        )
                    nc.vector.tensor_copy(out=res[:], in_=bi[:])

            # --- readout ---
            for c in range(C):
                pro = ppool.tile([JC, OUT], F32, space="PSUM", tag="mm",
                                 name=f"pro{c}")
                for h in range(2):
                    nc.tensor.matmul(
                        pro[:],
                        res_t[c][:, h * JC:(h + 1) * JC],
                        rwt_t[h][:],
                        start=(h == 0),
                        stop=(h == 1),
                    )
                ro = wpool.tile([JC, OUT], F32, tag=f"ro{c}", name=f"ro{c}")
                nc.vector.tensor_tensor(
                    out=ro[:], in0=pro[:], in1=bias_t[:], op=AluOpType.add,
                )
                nc.sync.dma_start(
                    out=out_d.ap()[c * JC:(c + 1) * JC, :], in_=ro[:],
                )

    nc.compile()
    return nc


def prep_inputs(x, lut, init_res, W, primes, input_nodes, readout_w, readout_b,
                steps: int = S):
    """Host-side prep: permutation, weight relayout, windowed LUT table, xc.

    Returns (shared_map, per_core_maps)."""
    x = np.asarray(x)
    lut = np.asarray(lut, dtype=np.int32)
    init_res = np.asarray(init_res)
    W = np.asarray(W)
    primes = np.asarray(primes, dtype=np.int64)
    input_nodes = np.asarray(input_nodes, dtype=np.int64)
    readout_w = np.asarray(readout_w, dtype=np.float32)
    readout_b = np.asarray(readout_b, dtype=np.float32)

    m, s, d, b = x.shape
    assert (m, s, d * b) == (M, S, NB) and steps <= S

    # idx (with input contributions) must fit the windowed table
    max_idx = int((W.astype(np.int64) * primes[None, :]).sum(1).max())
    assert max_idx < (1 << (WSHIFT + 7)), max_idx

    others = np.array(sorted(set(range(R)) - set(input_nodes.tolist())),
                      dtype=np.int64)
    nodes_at = np.concatenate([others, input_nodes])  # position -> orig node id

    # PW with input-node rows (contraction side) zeroed, permuted, lhsT chunks
    c_mat = (W.astype(np.int64) * primes[None, :]).astype(np.float64)
    c_mat[:, input_nodes] = 0.0
    cp = c_mat[np.ix_(nodes_at, nodes_at)]
    pw = np.zeros((2, 2, P_FULL, P_FULL), dtype=np.float32)
    for ho in range(2):
        for hk in range(2):
            pw[ho, hk] = cp[ho::2, hk::2].T.astype(np.float32)

    # windowed LUT table: row n'*128 + w = bytes [128w, 128w+256) of node n'
    lb = np.packbits(lut[nodes_at].astype(np.uint8), axis=1, bitorder="little")
    lb = np.ascontiguousarray(lb)  # [256, 32768] bytes
    st = lb.strides
    tab_u8 = np.lib.stride_tricks.as_strided(
        lb, shape=(R, NWIN, 256), strides=(st[0], 128 * st[1], st[1]))
    tab = np.ascontiguousarray(tab_u8).reshape(R * NWIN, 256)
    tab = tab.view(np.int32).reshape(NROWS, EW)

    # per-position gather row base: (2p+h)*128
    n_of = (2 * np.arange(P_FULL)[:, None]
            + (np.arange(FBC)[None, :] // JC))        # [128, FBC] -> n'
    baseg = (n_of * NWIN).astype(np.int32)

    iota = np.broadcast_to(np.arange(WSEL, dtype=np.int32),
                           (P_FULL, FBC, WSEL)).reshape(P_FULL, FBC * WSEL)
    iota = np.ascontiguousarray(iota)

    # res0[p, c*FBC + h*JC + j] = init_res[nodes_at[2p+h]]
    r0 = init_res[nodes_at].astype(np.float32)
    res0 = np.broadcast_to(
        r0.reshape(P_FULL, 2)[:, None, :, None], (P_FULL, C, 2, JC)
    ).reshape(P_FULL, C * FBC).copy()

    # readout weights by position
    rwp = readout_w[:, nodes_at]
    rwt = np.stack([rwp[:, h::2].T for h in range(2)]).astype(np.float32)
    bias = np.broadcast_to(readout_b[None, :], (JC, OUT)).astype(np.float32).copy()

    # xc[t, m, n'] = input-bit contribution to idx (exact in fp32)
    xt = x.reshape(M, S, NB).astype(np.float32)
    cin = (primes[input_nodes][:, None]
           * W[:, input_nodes].astype(np.int64).T).astype(np.float32)
    xc_full = xt.reshape(M * S, NB) @ cin
    xc_full = xc_full.reshape(M, S, R)[:, :, nodes_at]

    nxc = (steps + XC_CHUNK - 1) // XC_CHUNK
    tpad = nxc * XC_CHUNK
    per_core = []
    for core in range(NCORES):
        xcc = xc_full[core * MLOC:(core + 1) * MLOC, :steps]
        if tpad != steps:
            xcc = np.concatenate(
                [xcc, np.zeros((MLOC, tpad - steps, R), xcc.dtype)], axis=1)
        # [m=(c,j), t=(ct,ti), n'=(p,h)] -> [ct, p, ti, c, h, j]
        arr = xcc.reshape(C, JC, nxc, XC_CHUNK, P_FULL, 2)
        arr = arr.transpose(2, 4, 3, 0, 5, 1)
        per_core.append({"xc": np.ascontiguousarray(
            arr.reshape(nxc, P_FULL, XC_CHUNK * C * FBC), dtype=np.float32)})

    shared = dict(tab=tab, pw=pw, rwt=rwt, bias=bias, baseg=baseg, iota=iota,
                  res0=res0)
    return shared, per_core


_NC_CACHE = {}


def _get_nc(steps=S):
    if steps not in _NC_CACHE:
        _NC_CACHE[steps] = build_bass(steps)
    return _NC_CACHE[steps]


def kernel(**inputs) -> np.ndarray:
    nc = _get_nc(S)
    shared, per_core = prep_inputs(**inputs)
    in_maps = [{**shared, **pc} for pc in per_core]
    res = run_bass_kernel_spmd(nc, in_maps, core_ids=list(range(NCORES)))
    out = np.concatenate([np.asarray(r["out"]) for r in res.results], axis=0)
    return out.astype(np.float32)
